# revision 20
# baseline (speedup 1.0000x reference)
"""Trainium2 Bass kernel for GNN message-passing attention MLP.

Computation (per node b with N=32 neighbors, F=128 features):
  h_nb   = relu(input1 @ W_nb + b_nb)          [B,N,H1]
  h_self = relu(input2 @ W_self + b_self)      [B,H1]
  z      = relu(h_nb @ W_a1[:H1] + h_self @ W_a1[H1:] + b_a1)   [B,N,H2]
  out    = (z @ W_a2 + b_a2).reshape(B*N, 1)

Strategy: data-parallel over 8 NeuronCores (6250 nodes each, padded to
6272).  Host-side prep casts inputs to bf16 and pre-transposes them to
[F, neighbor, node] layout so the contraction dim lands on SBUF
partitions, and packs the tiny weights into PE-friendly
stacked/replicated matrices.  On device, nodes ride the matmul free dim
(512-wide superblocks); per-neighbor z tiles are accumulated directly in
PSUM (self-path + bias via a replicated-weight matmul), and the final
H2-contraction is done as K=128 matmuls with zero-padded weights that
gather all 32 neighbors into one [32, nodes] PSUM tile.
"""

import numpy as np
import ml_dtypes

import concourse.bass as bass
import concourse.mybir as mybir
from concourse import bacc
from concourse.tile import TileContext
from concourse.bass_utils import run_bass_kernel_spmd

BF16 = ml_dtypes.bfloat16

B, N, F = 50000, 32, 128
H1, H2 = 64, 16
N_CORES = 8
B_SH = B // N_CORES            # 6250 nodes per core
B_PAD = 6272                   # padded to 49*128
SB = 512                       # superblock: nodes per compute block
SBS = [(s * SB, SB) for s in range(B_PAD // SB)]
_rem = B_PAD - (B_PAD // SB) * SB
if _rem:
    SBS.append(((B_PAD // SB) * SB, _rem))
R_PAD = B_PAD * N              # padded rows per core (200704)
R_SH = B_SH * N                # valid rows per core (200000)

_cache = {}
last_results = None  # BassKernelResults of the most recent run (for test harness)
TRACE = False        # set True from test harness to capture an HW profile


def _build_graph():
    dt = mybir.dt
    nc = bacc.Bacc("TRN2", target_bir_lowering=False, debug=False,
                   num_devices=N_CORES)

    xt = nc.declare_dram_parameter("xt", [128 * N * B_PAD], dt.bfloat16, isOutput=False)
    x2t = nc.declare_dram_parameter("x2t", [128 * B_PAD], dt.bfloat16, isOutput=False)
    wnb = nc.declare_dram_parameter("wnb", [128, H1], dt.bfloat16, isOutput=False)
    wself = nc.declare_dram_parameter("wself", [128, H1], dt.bfloat16, isOutput=False)
    w2a = nc.declare_dram_parameter("w2a", [128, 32], dt.bfloat16, isOutput=False)
    wrep = nc.declare_dram_parameter("wrep", [H1, 128], dt.bfloat16, isOutput=False)
    wg = nc.declare_dram_parameter("wg", [8, 128, 32], dt.bfloat16, isOutput=False)
    bnb = nc.declare_dram_parameter("bnb", [128, 1], dt.float32, isOutput=False)
    bself = nc.declare_dram_parameter("bself", [H1, 1], dt.float32, isOutput=False)
    bz = nc.declare_dram_parameter("bz", [128, 1], dt.float32, isOutput=False)
    ident = nc.declare_dram_parameter("ident", [32, 32], dt.float32, isOutput=False)
    out = nc.declare_dram_parameter("out", [R_PAD], dt.float32, isOutput=True)

    RELU = mybir.ActivationFunctionType.Relu

    with TileContext(nc) as tc:
        with tc.tile_pool(name="const", bufs=1) as cpool, \
             tc.tile_pool(name="xp", bufs=3) as xpool, \
             tc.tile_pool(name="hp", bufs=10) as hpool, \
             tc.tile_pool(name="zs", bufs=10) as zpool, \
             tc.tile_pool(name="wp", bufs=2) as wpool, \
             tc.tile_pool(name="psum", bufs=1, space="PSUM") as ppool:
            # PSUM budget (8 banks): hp x4 (shared with warmup/self/wn),
            # zp x3, wa x1.

            wnb_sb = cpool.tile([128, H1], dt.bfloat16)
            nc.scalar.dma_start(out=wnb_sb[:], in_=wnb[:])
            wself_sb = cpool.tile([128, H1], dt.bfloat16)
            nc.scalar.dma_start(out=wself_sb[:], in_=wself[:])
            w2a_sb = cpool.tile([128, 32], dt.bfloat16)
            nc.scalar.dma_start(out=w2a_sb[:], in_=w2a[:])
            wrep_sb = cpool.tile([H1, 128], dt.bfloat16)
            nc.scalar.dma_start(out=wrep_sb[:], in_=wrep[:])
            wg_sb = cpool.tile([128, 8, 32], dt.bfloat16)
            nc.scalar.dma_start(out=wg_sb[:], in_=wg.rearrange("g p m -> p g m"))
            bnb_sb = cpool.tile([128, 1], dt.float32)
            nc.scalar.dma_start(out=bnb_sb[:], in_=bnb[:])
            bself_sb = cpool.tile([H1, 1], dt.float32)
            nc.scalar.dma_start(out=bself_sb[:], in_=bself[:])
            bz_sb = cpool.tile([128, 1], dt.float32)
            nc.scalar.dma_start(out=bz_sb[:], in_=bz[:])
            ident_sb = cpool.tile([32, 32], dt.float32)
            nc.scalar.dma_start(out=ident_sb[:], in_=ident[:])

            first = True
            pend_tail = None  # deferred output assembly of the previous sb

            def emit_tail(t):
                # transpose [32, ns] -> [ns/128 x [128, 32]] and store.
                # Runs early in the NEXT superblock so the PE never stalls
                # at the sb boundary waiting on the wa copy.
                wa_sb, tn0, tns, tCH = t
                wn_psum = ppool.tile([128, SB], dt.float32, tag="zp", bufs=4,
                                     name="wn_psum")
                for c in range(tCH):
                    nc.tensor.transpose(wn_psum[:, 32 * c: 32 * (c + 1)],
                                        wa_sb[:, 128 * c: 128 * (c + 1)],
                                        ident_sb[:])
                wout_sb = wpool.tile([128, 4, 32], dt.float32, tag="wo")
                nc.vector.tensor_copy(out=wout_sb[:, :tCH, :],
                                      in_=wn_psum[:, :32 * tCH].rearrange(
                                          "p (c j) -> p c j", j=32))
                nc.sync.dma_start(
                    out=out[tn0 * N: (tn0 + tns) * N].rearrange(
                        "(c p j) -> p c j", p=128, j=32),
                    in_=wout_sb[:, :tCH, :],
                )

            for n0, ns in SBS:
                CH = ns // 128  # 128-node output chunks in this superblock

                # -- inputs for this superblock (pre-transposed on host),
                #    split into 4 chunks of 8 neighbors so compute can start
                #    as soon as the first chunk lands --
                x2_sb = xpool.tile([128, SB], dt.bfloat16, tag="x2")
                nc.sync.dma_start(
                    out=x2_sb[:, :ns],
                    in_=x2t[128 * n0: 128 * (n0 + ns)].rearrange(
                        "(f n) -> f n", f=128),
                )
                x_sb = xpool.tile([128, N * SB], dt.bfloat16, tag="x")
                xt_sb = xt[128 * N * n0: 128 * N * (n0 + ns)].rearrange(
                    "(f j n) -> f j n", f=128, j=N)
                if first:
                    # First superblock: land neighbor 0 quickly so the HAM
                    # warm-up (which reads it) starts as early as possible,
                    # then j1-7, then the rest in two chunks.
                    jsplits = [(0, 1), (1, 8), (8, 20), (20, 32)]
                else:
                    jsplits = [(0, 16), (16, 32)]
                for j0, j1 in jsplits:
                    nc.sync.dma_start(
                        out=x_sb[:, j0 * ns: j1 * ns].rearrange(
                            "p (j n) -> p j n", j=j1 - j0),
                        in_=xt_sb[:, j0: j1, :],
                    )

                if first:
                    # HAM warm-up: ~3.5us of dense matmul right after the
                    # first DMA lands, so the PE clock-gate opens to 2.4GHz
                    # before the real stream starts.
                    first = False
                    warm = ppool.tile([128, SB], dt.float32, tag="hp", bufs=4)
                    for _ in range(8):
                        nc.tensor.matmul(warm[0:H1, :], wnb_sb[:],
                                         x_sb[:, :SB], start=True, stop=True)

                # -- self path: h_self = relu(W_self.T @ x2T + b_self) --
                hs_psum = ppool.tile([128, SB], dt.float32, tag="hp", bufs=4)
                nc.tensor.matmul(hs_psum[0:H1, :ns], wself_sb[:], x2_sb[:, :ns],
                                 start=True, stop=True)
                hself_sb = hpool.tile([H1, SB], dt.bfloat16, tag="hself")
                nc.scalar.activation(hself_sb[:, :ns], hs_psum[0:H1, :ns], RELU,
                                     bias=bself_sb[:], scale=1.0)

                # -- neighbor path, 8 groups of 4 neighbors, software
                #    pipelined with a 1-block (2-group) skew so the PE never
                #    waits on the relu engines --
                z_sbs = []
                pend = None

                def flush_blk(blk, ns=ns, z_sbs=z_sbs):
                    # mm2s first (start=True, M=32 with zero-padded columns
                    # so all 128 partitions are initialized), then the two
                    # z_self matmuls back-to-back (wrep LDW elided on the
                    # second), then the z relus.  The z accumulator's first
                    # write sits a full block after its relu-freed slot, so
                    # slot recycling stays off the PE critical path.
                    zps = []
                    for hs in blk:
                        zp = ppool.tile([128, SB], dt.float32, tag="zp",
                                        bufs=4, name="zp")
                        for pair in range(2):
                            for c in range(2):
                                jc = 2 * pair + c
                                nc.tensor.matmul(
                                    zp[32 * jc: 32 * jc + 32, :ns],
                                    w2a_sb[H1 * c: H1 * (c + 1), :],
                                    hs[pair][H1 * c: H1 * (c + 1), :ns],
                                    start=True, stop=False,
                                    skip_group_check=True,
                                    tile_position=(H1 * c, 32 * jc),
                                )
                        zps.append(zp)
                    for zp in zps:
                        # z_self replicated to the 4 column strips (+ the
                        # b_a2 enable slot via bz), zeros elsewhere
                        nc.tensor.matmul(zp[:, :ns], wrep_sb[:],
                                         hself_sb[:, :ns],
                                         start=False, stop=True,
                                         skip_group_check=True)
                    for zp in zps:
                        z_sb = zpool.tile([128, SB], dt.bfloat16, tag="z")
                        nc.scalar.activation(z_sb[:, :ns], zp[:, :ns], RELU,
                                             bias=bz_sb[:], scale=1.0)
                        z_sbs.append(z_sb)

                # Process groups in 2-group blocks: the mm1s of a block and
                # (via the 1-block skew) its mm2s are each emitted
                # back-to-back on the PE so walrus elides repeated LDWEIGHTS.
                for gg in range(0, 8, 2):
                    blk = []
                    for q in range(2):
                        g = gg + q
                        hs = []
                        for pair in range(2):
                            hp = ppool.tile([128, SB], dt.float32, tag="hp",
                                            bufs=4)
                            for c in range(2):
                                j = 4 * g + 2 * pair + c
                                nc.tensor.matmul(
                                    hp[H1 * c: H1 * (c + 1), :ns],
                                    wnb_sb[:],
                                    x_sb[:, j * ns: (j + 1) * ns],
                                    start=True, stop=True,
                                    tile_position=(0, H1 * c),
                                )
                            h_sb = hpool.tile([128, SB], dt.bfloat16, tag="h")
                            if (2 * g + pair) % 4 == 3:
                                nc.scalar.activation(h_sb[:, :ns], hp[:, :ns],
                                                     RELU, bias=bnb_sb[:],
                                                     scale=1.0)
                            else:
                                nc.vector.tensor_scalar(
                                    h_sb[:, :ns], hp[:, :ns],
                                    bnb_sb[:], 0.0,
                                    mybir.AluOpType.add, mybir.AluOpType.max)
                            hs.append(h_sb)
                        blk.append(hs)
                    if gg == 2 and pend_tail is not None:
                        emit_tail(pend_tail)
                        pend_tail = None
                    if pend is not None:
                        flush_blk(pend)
                    pend = blk
                flush_blk(pend)
                pend = None

                # -- final contraction: all 32 neighbors into [32, ns] --
                wa_psum = ppool.tile([32, SB], dt.float32, tag="zp", bufs=4,
                                     name="wa_psum")
                for g in range(8):
                    nc.tensor.matmul(wa_psum[:, :ns], wg_sb[:, g, :],
                                     z_sbs[g][:, :ns],
                                     start=(g == 0), stop=(g == 7),
                                     skip_group_check=True)
                wa_sb = wpool.tile([32, SB], dt.float32, tag="was")
                nc.vector.tensor_copy(out=wa_sb[:, :ns], in_=wa_psum[:, :ns])
                pend_tail = (wa_sb, n0, ns, CH)

            emit_tail(pend_tail)

    nc.compile()
    return nc


def _prep_weights(W_nb, b_nb, W_self, b_self, W_a1, b_a1, W_a2, b_a2):
    """Pack the dense weights into the layouts the kernel expects."""
    W_a1a = W_a1[:H1]          # [64, 16]
    W_a1b = W_a1[H1:]          # [64, 16]

    w2a = np.zeros((128, 32), np.float32)                     # [128, 32]
    w2a[:H1, :H2] = W_a1a
    w2a[H1:, :H2] = W_a1a

    wrep = np.zeros((H1, 128), np.float32)                    # [64, 128]
    for jc in range(4):
        wrep[:, 32 * jc: 32 * jc + H2] = W_a1b

    wg = np.zeros((8, 128, 32), np.float32)
    for g in range(8):
        for jc in range(4):
            wg[g, 32 * jc: 32 * jc + H2, 4 * g + jc] = W_a2[:, 0]
            wg[g, 16, 4 * g + jc] = b_a2[0]

    bnb = np.concatenate([b_nb, b_nb]).reshape(128, 1).astype(np.float32)
    bselfv = b_self.reshape(H1, 1).astype(np.float32)
    bzv = np.zeros((128, 1), np.float32)
    for jc in range(4):
        bzv[32 * jc: 32 * jc + H2, 0] = b_a1
    bzv[16, 0] = 1.0           # z==1 slot that carries b_a2 through wg

    return {
        "wnb": np.asarray(W_nb, np.float32).astype(BF16),
        "wself": np.asarray(W_self, np.float32).astype(BF16),
        "w2a": w2a.astype(BF16),
        "wrep": wrep.astype(BF16),
        "wg": wg.astype(BF16),
        "bnb": bnb,
        "bself": bselfv,
        "bz": bzv,
        "ident": np.eye(32, dtype=np.float32),
    }


def _prep_core_inputs(x_core, x2_core):
    """Pad to B_PAD nodes, cast to bf16, pre-transpose to [F, nbr, node]
    per superblock, and flatten."""
    xp = np.zeros((B_PAD, N, F), BF16)
    xp[:B_SH] = x_core.astype(BF16)
    x2p = np.zeros((B_PAD, F), BF16)
    x2p[:B_SH] = x2_core.astype(BF16)

    xt_parts = []
    x2t_parts = []
    for n0, ns in SBS:
        xt_parts.append(np.ascontiguousarray(
            xp[n0:n0 + ns].transpose(2, 1, 0)).reshape(-1))
        x2t_parts.append(np.ascontiguousarray(
            x2p[n0:n0 + ns].transpose(1, 0)).reshape(-1))
    return np.concatenate(xt_parts), np.concatenate(x2t_parts)


def kernel(input1, input2, W_nb, b_nb, W_self, b_self, W_a1, b_a1, W_a2, b_a2):
    global last_results
    if "nc" not in _cache:
        _cache["nc"] = _build_graph()
    nc = _cache["nc"]

    input1 = np.asarray(input1, np.float32)
    input2 = np.asarray(input2, np.float32)
    wmap = _prep_weights(
        np.asarray(W_nb, np.float32), np.asarray(b_nb, np.float32),
        np.asarray(W_self, np.float32), np.asarray(b_self, np.float32),
        np.asarray(W_a1, np.float32), np.asarray(b_a1, np.float32),
        np.asarray(W_a2, np.float32), np.asarray(b_a2, np.float32))

    in_maps = []
    for c in range(N_CORES):
        xt_c, x2t_c = _prep_core_inputs(
            input1[c * B_SH: (c + 1) * B_SH],
            input2[c * B_SH: (c + 1) * B_SH])
        m = dict(wmap)
        m["xt"] = xt_c
        m["x2t"] = x2t_c
        in_maps.append(m)

    res = run_bass_kernel_spmd(nc, in_maps, core_ids=list(range(N_CORES)),
                               trace=TRACE)
    last_results = res

    out = np.empty((B * N, 1), np.float32)
    for c in range(N_CORES):
        out[c * R_SH: (c + 1) * R_SH, 0] = res.results[c]["out"][:R_SH]
    return out
